# revision 11
# baseline (speedup 1.0000x reference)
"""Trainium2 Bass kernel for nn_CrossAttention_16690242912719.

Cross-attention with a dim=0 (batch-axis) softmax and v == k:
    q  = heads(x @ Wqk.T + bqk)
    k  = v = heads(y @ Wqk.T + bqk)
    z  = (q @ k.T) * Dh**-0.5            # (B,H,N,L)
    attn = softmax(z, axis=0)            # over the batch axis (B=4)
    out  = (attn @ v) @ Wproj.T + bproj
    returns (out, attn)

Sharding: tensor-parallel over heads. 16 heads / 8 cores = 2 heads/core =
a 128-feature slice of the qk projection. The batch softmax is local per
head shard. The final projection contracts over all 1024 hidden features,
so each core computes a partial product with its 128-column slice of
Wproj; the host sums the 8 partials (the unshard step) and adds bproj.

On-device layout is feature-major ([feat, token]) so the Dh=64
contraction of attention lands on the partition axis. Matmuls run as
fp32r (FP32 datapath, FP22 multiplies, 1 cycle/row at moving>=256); the
attn @ v stage runs in bf16 with its attn operand produced by XBAR
DMA-transposes ([n,l] -> [l,n]) so no compute engine pays for the
orientation flip that the partition-axis contraction requires.
"""

import sys

sys.path.insert(0, "/opt/trn_rl_repo")

import numpy as np  # noqa: E402

B, N, L, DIM = 4, 1024, 1024, 1024
HEADS, DH = 16, 64
NCORES = 8
HPC = HEADS // NCORES          # heads per core (2)
FPC = DIM // NCORES            # qk feature slice per core (128)
T = B * N                      # flattened tokens (4096)
SCALE = DH ** -0.5

# io_bf16: ship x/y (and qk weights) to the device in bf16, write attn and
# the partial projection back in bf16. Halves HBM traffic; rel-err goes
# from ~2e-4 (pure fp32r) to ~3e-3 (bf16 input quantization).
CONFIG = dict(io_bf16=False)

_PROG = {}


def _build_program(io_bf16):
    import concourse.mybir as mybir
    from concourse import bacc
    from concourse.tile import TileContext

    F32 = mybir.dt.float32
    F32R = mybir.dt.float32r
    BF16 = mybir.dt.bfloat16
    Act = mybir.ActivationFunctionType

    IN_DT = BF16 if io_bf16 else F32R      # x/y/Wqk stream + P1 matmul dtype
    AT_DT = BF16 if io_bf16 else F32       # attn DRAM dtype
    HID_DT = BF16 if io_bf16 else F32R     # hidden + Wproj + P3 dtype
    PO_DT = BF16 if io_bf16 else F32       # pout DRAM dtype

    nc = bacc.Bacc("TRN2", target_bir_lowering=False, debug=False,
                   num_devices=NCORES)

    xT = nc.declare_dram_parameter("xT", [DIM, T], IN_DT, isOutput=False)
    yT = nc.declare_dram_parameter("yT", [DIM, T], IN_DT, isOutput=False)
    wqk = nc.declare_dram_parameter("wqk", [DIM, FPC], IN_DT, isOutput=False)
    bqk = nc.declare_dram_parameter("bqk", [FPC, 1], F32, isOutput=False)
    wproj = nc.declare_dram_parameter("wproj", [FPC, DIM], HID_DT, isOutput=False)
    ident = nc.declare_dram_parameter("ident", [128, 64], F32R, isOutput=False)
    ident_f32 = nc.declare_dram_parameter("ident_f32", [128, 128], F32, isOutput=False)
    attn_o = nc.declare_dram_parameter("attn", [B, HPC, N, L], AT_DT, isOutput=True)
    pout_o = nc.declare_dram_parameter("pout", [T, DIM], PO_DT, isOutput=True)

    with TileContext(nc) as tc:
        with (
            tc.tile_pool(name="persist", bufs=1) as pers,
            tc.tile_pool(name="stream", bufs=4) as stream,
            tc.tile_pool(name="soft", bufs=4) as soft,
            tc.tile_pool(name="psum", bufs=4, space="PSUM") as psum,
        ):
            # ---- persistent SBUF tensors ----
            q_sb = pers.tile([FPC, T], F32R, tag="q")
            k_sb = pers.tile([FPC, T], F32R, tag="k")
            v_sb = pers.tile([128, B * HPC * 8 * 64], BF16, tag="v")
            hid_sb = [pers.tile([64, T], HID_DT, tag=f"hid{h}", name=f"hid{h}_sb")
                      for h in range(HPC)]
            wqk_sb = pers.tile([128, DIM], IN_DT, tag="wqk")
            wp_sb = [pers.tile([64, DIM], HID_DT, tag=f"wp{h}", name=f"wp{h}_sb")
                     for h in range(HPC)]
            bqk_sb = pers.tile([FPC, 1], F32, tag="bqk")
            id_sb = pers.tile([128, 64], F32R, tag="id")
            id128_sb = pers.tile([128, 128], F32, tag="id128")
            nc.sync.dma_start(id128_sb, ident_f32.ap())

            for kt in range(8):
                nc.sync.dma_start(wqk_sb[:, kt * 128:(kt + 1) * 128],
                                  wqk.ap()[kt * 128:(kt + 1) * 128, :])
            for h in range(HPC):
                nc.sync.dma_start(wp_sb[h], wproj.ap()[h * 64:(h + 1) * 64, :])
            nc.sync.dma_start(bqk_sb, bqk.ap())
            nc.sync.dma_start(id_sb, ident.ap())

            def vslot(b, h, lt):
                return ((b * HPC + h) * 8 + lt) * 64

            # ---- P1: projections (k first so attention can start early) ----
            for src, dst in ((yT, k_sb), (xT, q_sb)):
                for ch in range(8):          # 512-token chunks of T
                    cs = slice(ch * 512, (ch + 1) * 512)
                    ps = psum.tile([128, 512], F32, tag="z", bufs=4)
                    for kt in range(8):
                        xt = stream.tile([128, 512], IN_DT, tag="xy", bufs=4,
                                         name="xt")
                        nc.sync.dma_start(xt, src.ap()[kt * 128:(kt + 1) * 128, cs])
                        nc.tensor.matmul(ps, wqk_sb[:, kt * 128:(kt + 1) * 128],
                                         xt, start=(kt == 0), stop=(kt == 7))
                    nc.scalar.activation(dst[:, cs], ps, Act.Identity,
                                         bias=bqk_sb[:, 0:1], scale=1.0)

            # ---- P1.5: v = k transposed to token-major (bf16) ----
            for b in range(B):
                for h in range(HPC):
                    hs = slice(h * 64, (h + 1) * 64)
                    tp = psum.tile([128, 512], F32R, tag="hid", bufs=4,
                                   name="tp")
                    for lt in range(8):
                        nc.tensor.transpose(
                            tp[:, lt * 64:(lt + 1) * 64],
                            k_sb[hs, b * L + lt * 128:b * L + (lt + 1) * 128],
                            id_sb[hs, 0:64])
                    nc.scalar.copy(v_sb[:, vslot(b, h, 0):vslot(b, h, 0) + 512], tp)

            # ---- P2: attention ----
            # A-side (per head, n-chunk): z = q.k in [n,l]; batch softmax;
            # attn tiles DMA'd out in the output's [n,l] layout; the
            # reciprocal tiles r = 1/sum_b exp(z) are kept in SBUF.
            # T-side: z recomputed in [l,n] orientation (same matmul with
            # swapped operands); r is re-oriented with 4 PE transposes per
            # tile (shared across the 4 batch entries) and consumed straight
            # from PSUM; attn.T = exp(z.T)*r.T feeds the attn @ v matmuls,
            # col-packed two batch entries per PSUM bank.
            for h in range(HPC):
                hs = slice(h * 64, (h + 1) * 64)
                for nc4 in range(2):
                    rtiles = {}
                    for lc in range(2):
                        for j in range(4):
                            nt = nc4 * 4 + j
                            zz = []
                            for b in range(B):
                                z = psum.tile([128, 512], F32, tag="z", bufs=4,
                                              name="z")
                                nc.tensor.matmul(
                                    z,
                                    q_sb[hs, b * N + nt * 128:b * N + (nt + 1) * 128],
                                    k_sb[hs, b * L + lc * 512:b * L + (lc + 1) * 512],
                                    start=True, stop=True)
                                zz.append(z)
                            ee = []
                            for b in range(B):
                                e = soft.tile([128, 512], F32, tag="e", bufs=8,
                                              name="e")
                                nc.scalar.activation(e, zz[b], Act.Exp, scale=SCALE)
                                ee.append(e)
                            s01 = soft.tile([128, 512], F32, tag="s", bufs=6, name="s01")
                            s23 = soft.tile([128, 512], F32, tag="s", bufs=6, name="s23")
                            ssum = soft.tile([128, 512], F32, tag="s", bufs=6, name="ssum")
                            rr = soft.tile([128, 512], F32, tag="r", bufs=8, name="rr")
                            nc.vector.tensor_add(s01, ee[0], ee[1])
                            nc.vector.tensor_add(s23, ee[2], ee[3])
                            nc.vector.tensor_add(ssum, s01, s23)
                            nc.vector.reciprocal_approx_fast(rr, ssum)
                            rtiles[(j, lc)] = rr
                            for b in range(B):
                                ao = soft.tile([128, 512], F32, tag="ao",
                                               bufs=6, name="ao")
                                nc.vector.tensor_mul(ao, ee[b], rr)
                                nc.sync.dma_start(
                                    attn_o.ap()[b, h,
                                                nt * 128:(nt + 1) * 128,
                                                lc * 512:(lc + 1) * 512],
                                    ao)
                    # T-side + attn @ v
                    hid_ps = [psum.tile([64, 512], F32, tag="hid", bufs=4,
                                        name="hid_ps") for _ in range(B)]
                    for lc in range(2):
                        for lb in range(4):
                            lt = lc * 4 + lb
                            rT = psum.tile([128, 512], F32, tag="z", bufs=4,
                                           name="rT")
                            for j in range(4):
                                nc.tensor.transpose(
                                    rT[:, j * 128:(j + 1) * 128],
                                    rtiles[(j, lc)][:, lb * 128:(lb + 1) * 128],
                                    id128_sb)
                            for b in range(B):
                                zt = psum.tile([128, 512], F32, tag="z", bufs=4,
                                               name="zt")
                                nc.tensor.matmul(
                                    zt,
                                    k_sb[hs, b * L + lt * 128:b * L + (lt + 1) * 128],
                                    q_sb[hs, b * N + nc4 * 512:b * N + (nc4 + 1) * 512],
                                    start=True, stop=True)
                                eT = soft.tile([128, 512], F32, tag="e", bufs=8,
                                               name="eT")
                                nc.scalar.activation(eT, zt, Act.Exp, scale=SCALE)
                                abfT = soft.tile([128, 512], BF16, tag="abf",
                                                 bufs=6, name="abfT")
                                nc.vector.tensor_mul(abfT, eT, rT)
                                nc.tensor.matmul(
                                    hid_ps[b],
                                    v_sb[:, vslot(b, h, lt):vslot(b, h, lt) + 64],
                                    abfT,
                                    start=(lt == 0), stop=(lt == 7))
                    for b in range(B):
                        nc.scalar.copy(hid_sb[h][:, b * N + nc4 * 512:
                                                 b * N + (nc4 + 1) * 512],
                                       hid_ps[b])

            # ---- P3: partial projection pout = hidden.T @ Wproj[:,F].T ----
            for mt in range(T // 128):
                ms = slice(mt * 128, (mt + 1) * 128)
                for oc in range(2):
                    ocs = slice(oc * 512, (oc + 1) * 512)
                    ps = psum.tile([128, 512], F32, tag="z", bufs=4, name="pp")
                    nc.tensor.matmul(ps, hid_sb[0][:, ms], wp_sb[0][:, ocs],
                                     start=True, stop=False)
                    nc.tensor.matmul(ps, hid_sb[1][:, ms], wp_sb[1][:, ocs],
                                     start=False, stop=True)
                    po = soft.tile([128, 512], PO_DT, tag="po", bufs=4, name="po")
                    nc.scalar.copy(po, ps)
                    nc.sync.dma_start(pout_o.ap()[ms, ocs], po)

    nc.compile()
    return nc


def _get_program(io_bf16):
    if io_bf16 not in _PROG:
        _PROG[io_bf16] = _build_program(io_bf16)
    return _PROG[io_bf16]


def _bf16(a):
    import ml_dtypes
    return a.astype(ml_dtypes.bfloat16)


def _make_in_maps(x, y, Wqk, bqk, Wproj, bproj, io_bf16):
    xT = np.ascontiguousarray(x.reshape(T, DIM).T)
    yT = np.ascontiguousarray(y.reshape(T, DIM).T)
    if io_bf16:
        xT, yT = _bf16(xT), _bf16(yT)
    import ml_dtypes
    ident = np.zeros((128, 64), np.float32)
    ident[0:64, :] = np.eye(64, dtype=np.float32)
    ident[64:128, :] = np.eye(64, dtype=np.float32)
    ident_f32 = np.eye(128, dtype=np.float32)
    in_maps = []
    for c in range(NCORES):
        fs = slice(c * FPC, (c + 1) * FPC)
        wqk_c = np.ascontiguousarray(Wqk[fs, :].T)
        wp_c = np.ascontiguousarray(Wproj[:, fs].T)
        if io_bf16:
            wqk_c, wp_c = _bf16(wqk_c), _bf16(wp_c)
        in_maps.append({
            "xT": xT,
            "yT": yT,
            "wqk": wqk_c,
            "bqk": np.ascontiguousarray(bqk[fs]).reshape(FPC, 1),
            "wproj": wp_c,
            "ident": ident,
            "ident_f32": ident_f32,
        })
    return in_maps


def _assemble(results, bproj):
    attn = np.empty((B, HEADS, N, L), np.float32)
    out = None
    for c in range(NCORES):
        attn[:, c * HPC:(c + 1) * HPC] = np.asarray(results[c]["attn"],
                                                    np.float32)
        p = np.asarray(results[c]["pout"], np.float32)
        out = p.copy() if out is None else out + p
    out = out + bproj[None, :]
    return out.reshape(B, N, DIM), attn


def kernel_ext(inputs, trace=False, trace_kwargs=None):
    """Run on the 8 NeuronCores; returns ((out, attn), BassKernelResults)."""
    from concourse.bass_utils import run_bass_kernel_spmd

    io_bf16 = CONFIG["io_bf16"]
    nc = _get_program(io_bf16)
    in_maps = _make_in_maps(**inputs, io_bf16=io_bf16)
    res = run_bass_kernel_spmd(nc, in_maps, list(range(NCORES)), trace=trace,
                               **(trace_kwargs or {}))
    out, attn = _assemble(res.results, np.asarray(inputs["bproj"], np.float32))
    return (out, attn), res


def kernel(x, y, Wqk, bqk, Wproj, bproj):
    x = np.asarray(x, np.float32)
    y = np.asarray(y, np.float32)
    Wqk = np.asarray(Wqk, np.float32)
    bqk = np.asarray(bqk, np.float32)
    Wproj = np.asarray(Wproj, np.float32)
    bproj = np.asarray(bproj, np.float32)
    (out, attn), _ = kernel_ext(dict(x=x, y=y, Wqk=Wqk, bqk=bqk,
                                     Wproj=Wproj, bproj=bproj))
    return out, attn


# revision 12
# speedup vs baseline: 1.2733x; 1.2733x over previous
"""Trainium2 Bass kernel for nn_CrossAttention_16690242912719.

Cross-attention with a dim=0 (batch-axis) softmax and v == k:
    q  = heads(x @ Wqk.T + bqk)
    k  = v = heads(y @ Wqk.T + bqk)
    z  = (q @ k.T) * Dh**-0.5            # (B,H,N,L)
    attn = softmax(z, axis=0)            # over the batch axis (B=4)
    out  = (attn @ v) @ Wproj.T + bproj
    returns (out, attn)

Sharding: tensor-parallel over heads. 16 heads / 8 cores = 2 heads/core =
a 128-feature slice of the qk projection. The batch softmax is local per
head shard. The final projection contracts over all 1024 hidden features,
so each core computes a partial product with its 128-column slice of
Wproj; the host sums the 8 partials (the unshard step) and adds bproj.

On-device layout is feature-major ([feat, token]) so the Dh=64
contraction of attention lands on the partition axis. fp32r matmuls
(FP32 datapath, FP22 multiplies) run 1 cycle/row; the two local heads'
K=64 attention matmuls are issued as adjacent row-group pairs
(partitions 0-63 / 64-127) so they execute concurrently in the PE array
and keep its activity monitor at the 2.4 GHz clock. attn is needed both
as [n,l] (the output) and [l,n] (operand of attn @ v, which contracts l
on the partition axis), so z is computed in both orientations; the
second softmax reuses the batch-sum reciprocals by PE-transposing them
(shared across the 4 batch entries). attn @ v runs in bf16 with the two
heads col-packed per (batch) PSUM bank (an explicit ordering dep keeps
the bank-clearing first matmul first).
"""

import sys

sys.path.insert(0, "/opt/trn_rl_repo")

import numpy as np  # noqa: E402

B, N, L, DIM = 4, 1024, 1024, 1024
HEADS, DH = 16, 64
NCORES = 8
HPC = HEADS // NCORES          # heads per core (2)
FPC = DIM // NCORES            # qk feature slice per core (128)
T = B * N                      # flattened tokens (4096)
SCALE = DH ** -0.5

# io_bf16: ship x/y (and qk weights) to the device in bf16, write attn and
# the partial projection back in bf16. Halves HBM traffic; rel-err goes
# from ~1e-3 to ~3e-3 (bf16 input quantization).
CONFIG = dict(io_bf16=False)

_PROG = {}


def _build_program(io_bf16):
    import concourse.bass as bass
    import concourse.mybir as mybir
    from concourse import bacc
    from concourse.tile import TileContext

    F32 = mybir.dt.float32
    F32R = mybir.dt.float32r
    BF16 = mybir.dt.bfloat16
    Act = mybir.ActivationFunctionType

    IN_DT = BF16 if io_bf16 else F32R      # x/y/Wqk stream + P1 matmul dtype
    AT_DT = BF16 if io_bf16 else F32       # attn DRAM dtype
    PO_DT = BF16 if io_bf16 else F32       # pout DRAM dtype

    nc = bacc.Bacc("TRN2", target_bir_lowering=False, debug=False,
                   num_devices=NCORES)

    xT = nc.declare_dram_parameter("xT", [DIM, T], IN_DT, isOutput=False)
    yT = nc.declare_dram_parameter("yT", [DIM, T], IN_DT, isOutput=False)
    wqk = nc.declare_dram_parameter("wqk", [DIM, FPC], IN_DT, isOutput=False)
    bqk = nc.declare_dram_parameter("bqk", [FPC, 1], F32, isOutput=False)
    wproj = nc.declare_dram_parameter("wproj", [FPC, DIM], F32R, isOutput=False)
    ident = nc.declare_dram_parameter("ident", [128, 64], F32R, isOutput=False)
    ident_f32 = nc.declare_dram_parameter("ident_f32", [128, 128], F32, isOutput=False)
    attn_o = nc.declare_dram_parameter("attn", [B, HPC, N, L], AT_DT, isOutput=True)
    pout_o = nc.declare_dram_parameter("pout", [T, DIM], PO_DT, isOutput=True)

    with TileContext(nc) as tc:
        with (
            tc.tile_pool(name="persist", bufs=1) as pers,
            tc.tile_pool(name="stream", bufs=4) as stream,
            tc.tile_pool(name="soft", bufs=4) as soft,
            tc.tile_pool(name="psum", bufs=4, space="PSUM") as psum,
        ):
            # ---- persistent SBUF tensors ----
            q_sb = pers.tile([FPC, T], F32R, tag="q")
            k_sb = pers.tile([FPC, T], F32R, tag="k")
            v_sb = pers.tile([128, B * HPC * 8 * 64], BF16, tag="v")
            # hidden, feature-major [(h,d) = 128 local features, (b,n)]
            hid_sb = pers.tile([128, T], F32R, tag="hid_sb")
            wqk_sb = pers.tile([128, DIM], IN_DT, tag="wqk")
            wp_sb = pers.tile([128, DIM], F32R, tag="wp")
            bqk_sb = pers.tile([FPC, 1], F32, tag="bqk")
            id_sb = pers.tile([128, 64], F32R, tag="id")
            id128_sb = pers.tile([128, 128], F32, tag="id128")

            for kt in range(8):
                nc.sync.dma_start(wqk_sb[:, kt * 128:(kt + 1) * 128],
                                  wqk.ap()[kt * 128:(kt + 1) * 128, :])
            nc.sync.dma_start(wp_sb, wproj.ap())
            nc.sync.dma_start(bqk_sb, bqk.ap())
            nc.sync.dma_start(id_sb, ident.ap())
            nc.sync.dma_start(id128_sb, ident_f32.ap())

            def vslot(b, h, lt):
                return ((b * HPC + h) * 8 + lt) * 64

            # ---- P1: projections (k first so attention can start early) ----
            for src, dst in ((yT, k_sb), (xT, q_sb)):
                for ch in range(8):          # 512-token chunks of T
                    cs = slice(ch * 512, (ch + 1) * 512)
                    ps = psum.tile([128, 512], F32, tag="z", bufs=4)
                    for kt in range(8):
                        xt = stream.tile([128, 512], IN_DT, tag="xy", bufs=12,
                                         name="xt")
                        nc.sync.dma_start(xt, src.ap()[kt * 128:(kt + 1) * 128, cs])
                        nc.tensor.matmul(ps, wqk_sb[:, kt * 128:(kt + 1) * 128],
                                         xt, start=(kt == 0), stop=(kt == 7))
                    nc.scalar.activation(dst[:, cs], ps, Act.Identity,
                                         bias=bqk_sb[:, 0:1], scale=1.0)

            # ---- P1.5: v = k transposed to token-major (bf16) ----
            for b in range(B):
                for h in range(HPC):
                    hs = slice(h * 64, (h + 1) * 64)
                    tp = psum.tile([128, 512], F32R, tag="hid", bufs=4,
                                   name="tp")
                    for lt in range(8):
                        nc.tensor.transpose(
                            tp[:, lt * 64:(lt + 1) * 64],
                            k_sb[hs, b * L + lt * 128:b * L + (lt + 1) * 128],
                            id_sb[hs, 0:64])
                    nc.scalar.copy(v_sb[:, vslot(b, h, 0):vslot(b, h, 0) + 512], tp)

            # ---- P2: attention ----
            for nc4 in range(2):
                # A-side: z = q.k in [n,l]; the two heads' K=64 matmuls are
                # issued adjacently (row groups 0-63 / 64-127, concurrent in
                # the array). Batch softmax per head; attn tiles DMA'd out;
                # the reciprocal tiles are kept for the T-side.
                rtiles = {}
                for lc in range(2):
                    for j in range(4):
                        nt = nc4 * 4 + j
                        zz = {}
                        for b in range(B):
                            for h in range(HPC):
                                hs = slice(h * 64, (h + 1) * 64)
                                z = psum.tile([128, 512], F32, tag="z", bufs=4,
                                              name="z")
                                nc.tensor.matmul(
                                    z,
                                    q_sb[hs, b * N + nt * 128:b * N + (nt + 1) * 128],
                                    k_sb[hs, b * L + lc * 512:b * L + (lc + 1) * 512],
                                    start=True, stop=True)
                                zz[(h, b)] = z
                        ee = {}
                        for b in range(B):
                            for h in range(HPC):
                                e = soft.tile([128, 512], F32, tag="e", bufs=10,
                                              name="e")
                                nc.scalar.activation(e, zz[(h, b)], Act.Exp,
                                                     scale=SCALE)
                                ee[(h, b)] = e
                        for h in range(HPC):
                            s01 = soft.tile([128, 512], F32, tag="s", bufs=6, name="s01")
                            s23 = soft.tile([128, 512], F32, tag="s", bufs=6, name="s23")
                            ssum = soft.tile([128, 512], F32, tag="s", bufs=6, name="ssum")
                            rr = soft.tile([128, 512], F32, tag="r", bufs=16, name="rr")
                            nc.vector.tensor_add(s01, ee[(h, 0)], ee[(h, 1)])
                            nc.vector.tensor_add(s23, ee[(h, 2)], ee[(h, 3)])
                            nc.vector.tensor_add(ssum, s01, s23)
                            nc.vector.reciprocal_approx_fast(rr, ssum)
                            rtiles[(h, j, lc)] = rr
                            for b in range(B):
                                ao = soft.tile([128, 512], AT_DT, tag="ao",
                                               bufs=8, name="ao")
                                nc.vector.tensor_mul(ao, ee[(h, b)], rr)
                                nc.sync.dma_start(
                                    attn_o.ap()[b, h,
                                                nt * 128:(nt + 1) * 128,
                                                lc * 512:(lc + 1) * 512],
                                    ao)
                # T-side: z.T via swapped-operand matmuls (h-paired);
                # attn.T = exp(z.T) * r.T with r.T from PE transposes
                # (consumed straight from PSUM); attn @ v accumulates with
                # the two heads col-packed per (batch) PSUM bank.
                hid_ps = [psum.tile([128, 512], F32, tag="hid", bufs=4,
                                    name="hid_ps") for _ in range(B)]
                first_mm = {}
                for lc in range(2):
                    for lb in range(4):
                        lt = lc * 4 + lb
                        rT = {}
                        for h in range(HPC):
                            rt = psum.tile([128, 512], F32, tag="z", bufs=4,
                                           name="rt")
                            for j in range(4):
                                nc.tensor.transpose(
                                    rt[:, j * 128:(j + 1) * 128],
                                    rtiles[(h, j, lc)][:, lb * 128:(lb + 1) * 128],
                                    id128_sb)
                            rT[h] = rt
                        for b in range(B):
                            abfT = {}
                            for h in range(HPC):
                                hs = slice(h * 64, (h + 1) * 64)
                                zt = psum.tile([128, 512], F32, tag="z", bufs=4,
                                               name="zt")
                                nc.tensor.matmul(
                                    zt,
                                    k_sb[hs, b * L + lt * 128:b * L + (lt + 1) * 128],
                                    q_sb[hs, b * N + nc4 * 512:b * N + (nc4 + 1) * 512],
                                    start=True, stop=True)
                                eT = soft.tile([128, 512], F32, tag="e", bufs=10,
                                               name="eT")
                                nc.scalar.activation(eT, zt, Act.Exp, scale=SCALE)
                                ab = soft.tile([128, 512], BF16, tag="abf",
                                               bufs=8, name="abfT")
                                nc.vector.tensor_mul(ab, eT, rT[h])
                                abfT[h] = ab
                            for h in range(HPC):
                                m = nc.tensor.matmul(
                                    hid_ps[b][h * 64:(h + 1) * 64, :],
                                    v_sb[:, vslot(b, h, lt):vslot(b, h, lt) + 64],
                                    abfT[h],
                                    start=(lt == 0 and h == 0),
                                    stop=(lt == 7 and h == 1),
                                    tile_position=(0, h * 64),
                                    skip_group_check=True)
                                # the (h==0, lt==0) matmul clears the bank's
                                # has_written bits; the other head's first
                                # matmul must stay behind it in PE order.
                                if lt == 0:
                                    if h == 0:
                                        first_mm[b] = m
                                    else:
                                        bass._add_dep_helper(
                                            m.ins, first_mm[b].ins, sync=False,
                                            reason="hid bank clear order")
                for b in range(B):
                    nc.scalar.copy(hid_sb[:, b * N + nc4 * 512:
                                          b * N + (nc4 + 1) * 512],
                                   hid_ps[b])

            # ---- P3: partial projection pout = hidden.T @ Wproj[:,F].T ----
            for mt in range(T // 128):
                ms = slice(mt * 128, (mt + 1) * 128)
                for oc in range(2):
                    ocs = slice(oc * 512, (oc + 1) * 512)
                    ps = psum.tile([128, 512], F32, tag="z", bufs=4, name="pp")
                    nc.tensor.matmul(ps, hid_sb[:, ms], wp_sb[:, ocs],
                                     start=True, stop=True)
                    po = soft.tile([128, 512], PO_DT, tag="po", bufs=4, name="po")
                    nc.scalar.copy(po, ps)
                    nc.sync.dma_start(pout_o.ap()[ms, ocs], po)

    nc.compile()
    return nc


def _get_program(io_bf16):
    if io_bf16 not in _PROG:
        _PROG[io_bf16] = _build_program(io_bf16)
    return _PROG[io_bf16]


def _bf16(a):
    import ml_dtypes
    return a.astype(ml_dtypes.bfloat16)


def _make_in_maps(x, y, Wqk, bqk, Wproj, bproj, io_bf16):
    xT = np.ascontiguousarray(x.reshape(T, DIM).T)
    yT = np.ascontiguousarray(y.reshape(T, DIM).T)
    if io_bf16:
        xT, yT = _bf16(xT), _bf16(yT)
    ident = np.zeros((128, 64), np.float32)
    ident[0:64, :] = np.eye(64, dtype=np.float32)
    ident[64:128, :] = np.eye(64, dtype=np.float32)
    ident_f32 = np.eye(128, dtype=np.float32)
    in_maps = []
    for c in range(NCORES):
        fs = slice(c * FPC, (c + 1) * FPC)
        wqk_c = np.ascontiguousarray(Wqk[fs, :].T)
        if io_bf16:
            wqk_c = _bf16(wqk_c)
        in_maps.append({
            "xT": xT,
            "yT": yT,
            "wqk": wqk_c,
            "bqk": np.ascontiguousarray(bqk[fs]).reshape(FPC, 1),
            "wproj": np.ascontiguousarray(Wproj[:, fs].T),
            "ident": ident,
            "ident_f32": ident_f32,
        })
    return in_maps


def _assemble(results, bproj):
    attn = np.empty((B, HEADS, N, L), np.float32)
    out = None
    for c in range(NCORES):
        attn[:, c * HPC:(c + 1) * HPC] = np.asarray(results[c]["attn"],
                                                    np.float32)
        p = np.asarray(results[c]["pout"], np.float32)
        out = p.copy() if out is None else out + p
    out = out + bproj[None, :]
    return out.reshape(B, N, DIM), attn


def kernel_ext(inputs, trace=False, trace_kwargs=None):
    """Run on the 8 NeuronCores; returns ((out, attn), BassKernelResults)."""
    from concourse.bass_utils import run_bass_kernel_spmd

    io_bf16 = CONFIG["io_bf16"]
    nc = _get_program(io_bf16)
    in_maps = _make_in_maps(**inputs, io_bf16=io_bf16)
    res = run_bass_kernel_spmd(nc, in_maps, list(range(NCORES)), trace=trace,
                               **(trace_kwargs or {}))
    out, attn = _assemble(res.results, np.asarray(inputs["bproj"], np.float32))
    return (out, attn), res


def kernel(x, y, Wqk, bqk, Wproj, bproj):
    x = np.asarray(x, np.float32)
    y = np.asarray(y, np.float32)
    Wqk = np.asarray(Wqk, np.float32)
    bqk = np.asarray(bqk, np.float32)
    Wproj = np.asarray(Wproj, np.float32)
    bproj = np.asarray(bproj, np.float32)
    (out, attn), _ = kernel_ext(dict(x=x, y=y, Wqk=Wqk, bqk=bqk,
                                     Wproj=Wproj, bproj=bproj))
    return out, attn
